# revision 6
# baseline (speedup 1.0000x reference)
"""Trainium2 Bass kernel for nn_Encoder_MLP (embedding gather + sum + 2-layer MLP tail).

Reference computation:
    x = where(gate_seq < 0, A, gate_seq)            # [B, T]   (inputs never negative)
    Wr = W1.reshape(T, V, HID)
    h  = Wr[arange(T)[None,:], x].sum(axis=1) + b1  # [B, HID]  gather B*T rows, sum over T
    h  = relu(h); h = relu(h @ W2 + b2); out = h @ W3 + b3

Sharding (8 cores): data-parallel over the batch axis, W1 fully replicated
(bf16, 512 MB/core in HBM). Core m owns batches [8m, 8m+8) and gathers all
T=256 positions for them: 2048 rows via 32 dma_gather calls (the int16 index
limit caps one call's window at 32768 rows = 8 positions x 4096 vocab, and a
core only has 8 batches x 8 positions = 64 indices per window). Calls round-
robin over the 4 SWDGE queues; per-queue desc-gen serializes at ~1.4us fixed
+ 6ns/idx per call, so the gather phase is ~8 rounds x ~1.8us ~= 15us.
The point of this layout: NO collective. The T-sharded version needed a
[64,256] ReduceScatter whose collectives-init barrier + ncfw wake + RS cost
95+us of a 115-137us exec (the barrier absorbs cross-core NEFF launch skew);
with no collective every core runs independently: measured 46.4us for the
fold+mask-matmul variant, bounded by ~16us prologue+Q7-library-fetch floor,
~15us gather desc-gen, then the reduce+tail.

TRANSPOSE_GATHER=True path: gathers with transpose=True so each row lands
with HID on partitions (out[p, c, i] = row elem c*128+p of idx i). num_idxs
must be %128, so each 64-idx call pads 64 trailing -1 (skipped; out cols
64:128 stay stale and are never read). idx order i = b*8 + j puts each
batch's 8 positions in contiguous columns, so one DVE tensor_reduce per
window ([128,2,8,8] -> [128,2,8], axis=X) does the position sum, a 5-level
DVE fold tree sums the 32 windows, and the tail MLP needs NO transposes at
all: relu(+b1T) -> 4 matmuls with W2 chunks as lhsT -> relu(+b2T) ->
2 matmuls + rank-1 bias matmul -> [8,256] out. Host concatenates per-core
outputs. Fallback TRANSPOSE_GATHER=False: non-transpose gathers (row i on
partition i) + 32 accumulating PE mask matmuls (mask[p,b] = p%8==b,
contraction over partitions 0:64) replacing the DVE folds, then the same
tail after a PE transpose of h.

Index layout: idx list position i lives at idx_tile[i%16, i//16]
(16-partition wrap, replicated x8 for the 8 Q7 cores). Window-local row
value = j*4096 + gate[8m + b, 8w + j]; the +j*4096 rebase is done on device
(ubias const + DVE add); the host only permutes/retypes gate_seq
(value-independent layout marshaling).
"""

import sys

import numpy as np

if "/opt/trn_rl_repo" not in sys.path:
    sys.path.insert(0, "/opt/trn_rl_repo")

B = 64
T = 256
V = 4096
HID = 256
OUT = 256
NCORES = 8
BPC = B // NCORES          # batches per core = 8
WIN_POS = 8                # positions per gather window (int16 limit: 8*4096 = 32768 rows)
NWIN = T // WIN_POS        # 32 windows per core
WIN_ROWS = WIN_POS * V     # 32768
NVALID = BPC * WIN_POS     # 64 valid indices per window
NQ = 4                     # SWDGE queues

TRANSPOSE_GATHER = True
# idx columns per window: transpose mode needs num_idxs=128 (%128 rule) with
# 64 trailing -1; non-transpose mode uses num_idxs=64.
IDXC = 8 if TRANSPOSE_GATHER else 4

_CACHE = {}


def _host_consts():
    import ml_dtypes

    # idx list position i = (col % IDXC)*16 + p%16 within a window's columns.
    p = np.arange(128)[:, None]
    col = np.arange(NWIN * IDXC)[None, :]
    i = (col % IDXC) * 16 + (p % 16)
    if TRANSPOSE_GATHER:
        # i = b*8 + j (batch-major); pad i >= 64 -> ubias 0 (gate_prep holds -1)
        ubias = np.where(i < NVALID, (i % WIN_POS) * V, 0)
    else:
        # i = j*8 + b (position-major)
        ubias = (i // BPC) * V
    ubias = np.ascontiguousarray(
        np.broadcast_to(ubias, (128, NWIN * IDXC)).astype(np.int16)
    )
    # mask[p, b] = 1 if p % 8 == b   (partitions 0..63; non-transpose reduce)
    mask = (np.arange(64)[:, None] % BPC == np.arange(BPC)[None, :]).astype(
        ml_dtypes.bfloat16
    )
    eye8 = np.eye(8, dtype=np.float32)
    return ubias, np.ascontiguousarray(mask), eye8


def _build_nc():
    import concourse.bacc as bacc
    import concourse.mybir as mybir
    import concourse.tile as tile

    f32 = mybir.dt.float32
    bf16 = mybir.dt.bfloat16
    i16 = mybir.dt.int16
    Relu = mybir.ActivationFunctionType.Relu
    add = mybir.AluOpType.add
    AxX = mybir.AxisListType.X

    ubias_np, mask_np, eye8_np = _host_consts()

    nc = bacc.Bacc(
        "TRN2",
        target_bir_lowering=False,
        debug=False,
        num_devices=NCORES,
        num_swdge_queues=NQ,
    )

    gate_prep_d = nc.dram_tensor(
        "gate_prep", [128, NWIN * IDXC], i16, kind="ExternalInput"
    )
    w1_d = nc.dram_tensor("w1", [T * V, HID], bf16, kind="ExternalInput")
    w2lh_d = nc.dram_tensor("w2lh", [128, 2, 2, 128], bf16, kind="ExternalInput")
    w3_d = nc.dram_tensor("w3", [HID, OUT], bf16, kind="ExternalInput")
    b1_d = nc.dram_tensor("b1t", [128, 2], f32, kind="ExternalInput")
    b2_d = nc.dram_tensor("b2t", [128, 2], f32, kind="ExternalInput")
    b3_d = nc.dram_tensor("b3", [1, OUT], bf16, kind="ExternalInput")
    out_d = nc.dram_tensor("out", [BPC, OUT], f32, kind="ExternalOutput")

    ubias_d = nc.inline_tensor(ubias_np, name="ubias_const")
    mask_d = nc.inline_tensor(mask_np, name="mask_const")
    eye_d = nc.inline_tensor(eye8_np, name="eye_const")

    # Issue the mlp ucode library load before any Tile-scheduled work so the
    # ~10us Q7 library fetch overlaps the NEFF prologue instead of stalling
    # the first dma_gather until ~16us.
    from concourse import library_config

    nc.gpsimd.load_library(library_config.mlp)

    with tile.TileContext(nc) as tc:
        with (
            tc.tile_pool(name="const", bufs=1) as const,
            tc.tile_pool(name="gat", bufs=1) as gat,
            tc.tile_pool(name="work", bufs=2) as work,
            tc.tile_pool(name="psum", bufs=1, space="PSUM") as psum,
        ):
            # ---- critical path: indices ----
            gp = const.tile([128, NWIN * IDXC], i16, tag="gp")
            nc.sync.dma_start(gp[:], gate_prep_d[:])
            ub = const.tile([128, NWIN * IDXC], i16, tag="ub")
            nc.sync.dma_start(ub[:], ubias_d[:])
            idx = const.tile([128, NWIN * IDXC], i16, tag="idx")
            nc.vector.tensor_tensor(idx[:], gp[:], ub[:], add)

            # ---- consts / weights preload (no deps; fills DMA idle time) ----
            w2lh_sb = const.tile([128, 2, 2, 128], bf16, tag="w2lh")
            nc.scalar.dma_start(w2lh_sb[:], w2lh_d[:])
            w3_sb = const.tile([128, 2, OUT], bf16, tag="w3")
            nc.scalar.dma_start(w3_sb[:], w3_d[:, :].rearrange("(k p) n -> p k n", p=128))
            b1_sb = const.tile([128, 2], f32, tag="b1")
            nc.scalar.dma_start(b1_sb[:], b1_d[:])
            b2_sb = const.tile([128, 2], f32, tag="b2")
            nc.scalar.dma_start(b2_sb[:], b2_d[:])
            b3_sb = const.tile([1, OUT], bf16, tag="b3")
            nc.scalar.dma_start(b3_sb[:], b3_d[:])
            ones8 = const.tile([1, BPC], bf16, tag="ones8")
            nc.vector.memset(ones8[:], 1.0)

            if TRANSPOSE_GATHER:
                # ---- transposed gathers + per-window segmented reduce ----
                red = work.tile([128, NWIN, 2, BPC], f32, tag="red")
                g_tiles = []
                for w in range(NWIN):
                    g = gat.tile([128, 2, 128], bf16, tag=f"g{w}")
                    nc.gpsimd.dma_gather(
                        g[:],
                        w1_d[w * WIN_ROWS : (w + 1) * WIN_ROWS, :],
                        idx[:, w * IDXC : (w + 1) * IDXC],
                        128,
                        NVALID,
                        HID,
                        transpose=True,
                        queue_num=w % NQ,
                    )
                    g_tiles.append(g)
                    # sum the 8 positions of each batch: [128,2,8,8] -> [128,2,8]
                    gv = g[:, :, 0:NVALID].rearrange("p c (b j) -> p c b j", j=WIN_POS)
                    nc.vector.tensor_reduce(red[:, w, :, :], gv, AxX, add)
                # fold tree over the 32 windows -> hT [128, 2, 8] (f32)
                f = red
                n = NWIN
                while n > 1:
                    h = n // 2
                    nf = work.tile([128, h, 2, BPC], f32, tag=f"fold{h}")
                    nc.vector.tensor_add(nf[:], f[:, 0:h, :, :], f[:, h:n, :, :])
                    f = nf
                    n = h
                hT = f  # [128, 1, 2, BPC]
                # relu(hT + b1T) per 128-chunk -> bf16
                hTr = work.tile([128, 2, BPC], bf16, tag="hTr")
                for c in range(2):
                    nc.scalar.activation(
                        hTr[:, c, :], hT[:, 0, c, :], Relu, bias=b1_sb[:, c : c + 1]
                    )
            else:
                # ---- plain gathers (row i -> partition i) + mask matmuls ----
                eye_sb = const.tile([8, 8], f32, tag="eye")
                nc.scalar.dma_start(eye_sb[:], eye_d[:])
                mask_sb = const.tile([64, BPC], bf16, tag="mask")
                nc.scalar.dma_start(mask_sb[:], mask_d[:])
                psum_part = psum.tile([BPC, HID], f32, tag="part")
                for w in range(NWIN):
                    g = gat.tile([128, 1, HID], bf16, tag=f"g{w}")
                    nc.gpsimd.dma_gather(
                        g[:],
                        w1_d[w * WIN_ROWS : (w + 1) * WIN_ROWS, :],
                        idx[:, w * IDXC : (w + 1) * IDXC],
                        NVALID,
                        NVALID,
                        HID,
                        queue_num=w % NQ,
                    )
                    nc.tensor.matmul(
                        psum_part[:],
                        mask_sb[:],
                        g[0:64, 0, :],
                        start=(w == 0),
                        stop=(w == NWIN - 1),
                    )
                h_sb = work.tile([BPC, HID], f32, tag="h")
                nc.vector.tensor_copy(h_sb[:], psum_part[:])
                eye_sb_ = eye_sb
                hTr = work.tile([128, 2, BPC], bf16, tag="hTr")
                for c in range(2):
                    p_hT = psum.tile([128, BPC], f32, tag=f"p_hT{c}")
                    nc.tensor.transpose(
                        p_hT[:], h_sb[:, c * 128 : (c + 1) * 128], eye_sb_[:]
                    )
                    nc.scalar.activation(
                        hTr[:, c, :], p_hT[:], Relu, bias=b1_sb[:, c : c + 1]
                    )

            # ---- tail: h2T = relu(W2T-chunks @ hTr + b2T), out = h2 @ W3 + b3
            h2Tr = work.tile([128, 2, BPC], bf16, tag="h2Tr")
            for c in range(2):
                p_h2T = psum.tile([128, BPC], f32, tag=f"p_h2T{c}")
                nc.tensor.matmul(
                    p_h2T[:], w2lh_sb[:, 0, c, :], hTr[:, 0, :], start=True, stop=False
                )
                nc.tensor.matmul(
                    p_h2T[:], w2lh_sb[:, 1, c, :], hTr[:, 1, :], start=False, stop=True
                )
                nc.scalar.activation(
                    h2Tr[:, c, :], p_h2T[:], Relu, bias=b2_sb[:, c : c + 1]
                )
            p_o = psum.tile([BPC, OUT], f32, tag="p_o")
            nc.tensor.matmul(p_o[:], h2Tr[:, 0, :], w3_sb[:, 0, :], start=True, stop=False)
            nc.tensor.matmul(p_o[:], h2Tr[:, 1, :], w3_sb[:, 1, :], start=False, stop=False)
            nc.tensor.matmul(p_o[:], ones8[:], b3_sb[:], start=False, stop=True)
            out_sb = work.tile([BPC, OUT], f32, tag="out_sb")
            nc.vector.tensor_copy(out_sb[:], p_o[:])
            nc.sync.dma_start(out_d[:], out_sb[:])

    nc.compile()
    return nc


def get_nc():
    if "nc" not in _CACHE:
        _CACHE["nc"] = _build_nc()
    return _CACHE["nc"]


def make_in_maps(gate_seq, W1, b1, W2, b2, W3, b3):
    """Shard/marshal the full inputs into per-core input maps (values untouched:
    pure slicing, transposition, retyping and tiling)."""
    gate_seq = np.asarray(gate_seq)
    import ml_dtypes

    W1 = np.ascontiguousarray(np.asarray(W1).astype(ml_dtypes.bfloat16))
    W2 = np.asarray(W2, dtype=np.float32)
    W3 = np.ascontiguousarray(np.asarray(W3).astype(ml_dtypes.bfloat16))
    b1 = np.asarray(b1, dtype=np.float32)
    b2 = np.asarray(b2, dtype=np.float32)
    b3 = np.asarray(b3, dtype=np.float32)

    # W2 chunked for lhsT use: w2lh[p, kc, nc, f] = W2[kc*128 + p, nc*128 + f]
    w2lh = np.ascontiguousarray(
        W2.reshape(2, 128, 2, 128).transpose(1, 0, 2, 3).astype(ml_dtypes.bfloat16)
    )
    b1t = np.ascontiguousarray(b1.reshape(2, 128).T)  # b1t[p, c] = b1[c*128 + p]
    b2t = np.ascontiguousarray(b2.reshape(2, 128).T)
    b3r = np.ascontiguousarray(b3[None, :].astype(ml_dtypes.bfloat16))

    # index-layout permutation (see module docstring)
    p16 = np.arange(16)[:, None]                     # [16, 1]
    col = np.arange(NWIN * IDXC)[None, :]            # [1, NWIN*IDXC]
    i = (col % IDXC) * 16 + p16                      # [16, NWIN*IDXC]
    if TRANSPOSE_GATHER:
        valid = i < NVALID
        b_idx = np.where(valid, i // WIN_POS, 0)
        t_idx = np.where(valid, (col // IDXC) * WIN_POS + i % WIN_POS, 0)
    else:
        valid = np.broadcast_to(True, i.shape)
        b_idx = i % BPC
        t_idx = (col // IDXC) * WIN_POS + i // BPC

    in_maps = []
    for m in range(NCORES):
        gs = gate_seq[m * BPC : (m + 1) * BPC, :]    # [8, 256]
        A = gs[b_idx, t_idx].astype(np.int16)        # [16, NWIN*IDXC]
        A = np.where(valid, A, np.int16(-1))
        gate_prep = np.ascontiguousarray(np.tile(A, (8, 1)))  # [128, NWIN*IDXC]
        in_maps.append(
            {
                "gate_prep": gate_prep,
                "w1": W1,
                "w2lh": w2lh,
                "w3": W3,
                "b1t": b1t,
                "b2t": b2t,
                "b3": b3r,
            }
        )
    return in_maps


def run(inputs, trace=False, **spmd_kwargs):
    from concourse.bass_utils import run_bass_kernel_spmd

    nc = get_nc()
    in_maps = make_in_maps(**inputs)
    res = run_bass_kernel_spmd(
        nc, in_maps, core_ids=list(range(NCORES)), trace=trace, **spmd_kwargs
    )
    out = np.concatenate([r["out"] for r in res.results], axis=0)
    return out, res


def kernel(**inputs) -> np.ndarray:
    out, _ = run(inputs, trace=False)
    return out


# revision 7
# speedup vs baseline: 1.1974x; 1.1974x over previous
"""Trainium2 Bass kernel for nn_Encoder_MLP (embedding gather + sum + 2-layer MLP tail).

Reference computation:
    x = where(gate_seq < 0, A, gate_seq)            # [B, T]   (inputs never negative)
    Wr = W1.reshape(T, V, HID)
    h  = Wr[arange(T)[None,:], x].sum(axis=1) + b1  # [B, HID]  gather B*T rows, sum over T
    h  = relu(h); h = relu(h @ W2 + b2); out = h @ W3 + b3

Sharding (8 cores): data-parallel over the batch axis, W1 fully replicated
(bf16, 512 MB/core in HBM). Core m owns batches [8m, 8m+8) and gathers all
T=256 positions for them: 2048 rows via 32 dma_gather calls (the int16 index
limit caps one call's window at 32768 rows = 8 positions x 4096 vocab, and a
core only has 8 batches x 8 positions = 64 indices per window). Calls round-
robin over the 4 SWDGE queues; per-queue desc-gen serializes at ~1.4us fixed
+ 6ns/idx per call, so the gather phase is ~8 rounds x ~1.8us ~= 15us.
The point of this layout: NO collective. The T-sharded version needed a
[64,256] ReduceScatter whose collectives-init barrier + ncfw wake + RS cost
95+us of a 115-137us exec (the barrier absorbs cross-core NEFF launch skew);
with no collective every core runs independently: measured 46.4us for the
fold+mask-matmul variant, bounded by ~16us prologue+Q7-library-fetch floor,
~15us gather desc-gen, then the reduce+tail.

TRANSPOSE_GATHER=True path: gathers with transpose=True so each row lands
with HID on partitions (out[p, c, i] = row elem c*128+p of idx i). num_idxs
must be %128, so each 64-idx call pads 64 trailing -1 (skipped; out cols
64:128 stay stale and are never read). idx order i = b*8 + j puts each
batch's 8 positions in contiguous columns, so one DVE tensor_reduce per
window ([128,2,8,8] -> [128,2,8], axis=X) does the position sum, a 5-level
DVE fold tree sums the 32 windows, and the tail MLP needs NO transposes at
all: relu(+b1T) -> 4 matmuls with W2 chunks as lhsT -> relu(+b2T) ->
2 matmuls + rank-1 bias matmul -> [8,256] out. Host concatenates per-core
outputs. Fallback TRANSPOSE_GATHER=False: non-transpose gathers (row i on
partition i) + 32 accumulating PE mask matmuls (mask[p,b] = p%8==b,
contraction over partitions 0:64) replacing the DVE folds, then the same
tail after a PE transpose of h.

Index layout: idx list position i lives at idx_tile[i%16, i//16]
(16-partition wrap, replicated x8 for the 8 Q7 cores). Window-local row
value = j*4096 + gate[8m + b, 8w + j]; the +j*4096 rebase is done on device
(ubias const + DVE add); the host only permutes/retypes gate_seq
(value-independent layout marshaling).
"""

import sys

import numpy as np

if "/opt/trn_rl_repo" not in sys.path:
    sys.path.insert(0, "/opt/trn_rl_repo")

B = 64
T = 256
V = 4096
HID = 256
OUT = 256
NCORES = 8
BPC = B // NCORES          # batches per core = 8
WIN_POS = 8                # positions per gather window (int16 limit: 8*4096 = 32768 rows)
NWIN = T // WIN_POS        # 32 windows per core
WIN_ROWS = WIN_POS * V     # 32768
NVALID = BPC * WIN_POS     # 64 valid indices per window
NQ = 4                     # SWDGE queues

TRANSPOSE_GATHER = False
# idx columns per window: transpose mode needs num_idxs=128 (%128 rule) with
# 64 trailing -1; non-transpose mode uses num_idxs=64.
IDXC = 8 if TRANSPOSE_GATHER else 4

_CACHE = {}


def _host_consts():
    import ml_dtypes

    # idx list position i = (col % IDXC)*16 + p%16 within a window's columns.
    p = np.arange(128)[:, None]
    col = np.arange(NWIN * IDXC)[None, :]
    i = (col % IDXC) * 16 + (p % 16)
    if TRANSPOSE_GATHER:
        # i = b*8 + j (batch-major); pad i >= 64 -> ubias 0 (gate_prep holds -1)
        ubias = np.where(i < NVALID, (i % WIN_POS) * V, 0)
    else:
        # i = j*8 + b (position-major)
        ubias = (i // BPC) * V
    ubias = np.ascontiguousarray(
        np.broadcast_to(ubias, (128, NWIN * IDXC)).astype(np.int16)
    )
    # mask[p, b] = 1 if p % 8 == b   (partitions 0..63; non-transpose reduce)
    mask = (np.arange(64)[:, None] % BPC == np.arange(BPC)[None, :]).astype(
        ml_dtypes.bfloat16
    )
    eye8 = np.eye(8, dtype=np.float32)
    return ubias, np.ascontiguousarray(mask), eye8


def _build_nc():
    import concourse.bacc as bacc
    import concourse.mybir as mybir
    import concourse.tile as tile

    f32 = mybir.dt.float32
    bf16 = mybir.dt.bfloat16
    i16 = mybir.dt.int16
    Relu = mybir.ActivationFunctionType.Relu
    add = mybir.AluOpType.add
    AxX = mybir.AxisListType.X

    ubias_np, mask_np, eye8_np = _host_consts()

    nc = bacc.Bacc(
        "TRN2",
        target_bir_lowering=False,
        debug=False,
        num_devices=NCORES,
        num_swdge_queues=NQ,
    )

    gate_prep_d = nc.dram_tensor(
        "gate_prep", [128, NWIN * IDXC], i16, kind="ExternalInput"
    )
    w1_d = nc.dram_tensor("w1", [T * V, HID], bf16, kind="ExternalInput")
    w2lh_d = nc.dram_tensor("w2lh", [128, 2, 2, 128], bf16, kind="ExternalInput")
    w3_d = nc.dram_tensor("w3", [HID, OUT], bf16, kind="ExternalInput")
    b1_d = nc.dram_tensor("b1t", [128, 2], f32, kind="ExternalInput")
    b2_d = nc.dram_tensor("b2t", [128, 2], f32, kind="ExternalInput")
    b3_d = nc.dram_tensor("b3", [1, OUT], bf16, kind="ExternalInput")
    out_d = nc.dram_tensor("out", [BPC, OUT], f32, kind="ExternalOutput")

    ubias_d = nc.inline_tensor(ubias_np, name="ubias_const")
    mask_d = nc.inline_tensor(mask_np, name="mask_const")
    eye_d = nc.inline_tensor(eye8_np, name="eye_const")

    # Issue the mlp ucode library load before any Tile-scheduled work so the
    # ~10us Q7 library fetch overlaps the NEFF prologue instead of stalling
    # the first dma_gather until ~16us.
    from concourse import library_config

    nc.gpsimd.load_library(library_config.mlp)

    with tile.TileContext(nc) as tc:
        with (
            tc.tile_pool(name="const", bufs=1) as const,
            tc.tile_pool(name="gat", bufs=1) as gat,
            tc.tile_pool(name="work", bufs=2) as work,
            tc.tile_pool(name="psum", bufs=1, space="PSUM") as psum,
        ):
            # ---- critical path: indices ----
            gp = const.tile([128, NWIN * IDXC], i16, tag="gp")
            nc.sync.dma_start(gp[:], gate_prep_d[:])
            ub = const.tile([128, NWIN * IDXC], i16, tag="ub")
            nc.sync.dma_start(ub[:], ubias_d[:])
            idx = const.tile([128, NWIN * IDXC], i16, tag="idx")
            nc.vector.tensor_tensor(idx[:], gp[:], ub[:], add)

            # ---- consts / weights preload (no deps; fills DMA idle time) ----
            w2lh_sb = const.tile([128, 2, 2, 128], bf16, tag="w2lh")
            nc.scalar.dma_start(w2lh_sb[:], w2lh_d[:])
            w3_sb = const.tile([128, 2, OUT], bf16, tag="w3")
            nc.scalar.dma_start(w3_sb[:], w3_d[:, :].rearrange("(k p) n -> p k n", p=128))
            b1_sb = const.tile([128, 2], f32, tag="b1")
            nc.scalar.dma_start(b1_sb[:], b1_d[:])
            b2_sb = const.tile([128, 2], f32, tag="b2")
            nc.scalar.dma_start(b2_sb[:], b2_d[:])
            b3_sb = const.tile([1, OUT], bf16, tag="b3")
            nc.scalar.dma_start(b3_sb[:], b3_d[:])
            ones8 = const.tile([1, BPC], bf16, tag="ones8")
            nc.vector.memset(ones8[:], 1.0)

            if TRANSPOSE_GATHER:
                # ---- transposed gathers + per-window segmented reduce ----
                red = work.tile([128, NWIN, 2, BPC], f32, tag="red")
                g_tiles = []
                for w in range(NWIN):
                    g = gat.tile([128, 2, 128], bf16, tag=f"g{w}")
                    nc.gpsimd.dma_gather(
                        g[:],
                        w1_d[w * WIN_ROWS : (w + 1) * WIN_ROWS, :],
                        idx[:, w * IDXC : (w + 1) * IDXC],
                        128,
                        NVALID,
                        HID,
                        transpose=True,
                        queue_num=w % NQ,
                    )
                    g_tiles.append(g)
                    # sum the 8 positions of each batch: [128,2,8,8] -> [128,2,8]
                    gv = g[:, :, 0:NVALID].rearrange("p c (b j) -> p c b j", j=WIN_POS)
                    nc.vector.tensor_reduce(red[:, w, :, :], gv, AxX, add)
                # fold tree over the 32 windows -> hT [128, 2, 8] (f32)
                f = red
                n = NWIN
                while n > 1:
                    h = n // 2
                    nf = work.tile([128, h, 2, BPC], f32, tag=f"fold{h}")
                    nc.vector.tensor_add(nf[:], f[:, 0:h, :, :], f[:, h:n, :, :])
                    f = nf
                    n = h
                hT = f  # [128, 1, 2, BPC]
                # relu(hT + b1T) per 128-chunk -> bf16
                hTr = work.tile([128, 2, BPC], bf16, tag="hTr")
                for c in range(2):
                    nc.scalar.activation(
                        hTr[:, c, :], hT[:, 0, c, :], Relu, bias=b1_sb[:, c : c + 1]
                    )
            else:
                # ---- plain gathers (row i -> partition i) + mask matmuls ----
                eye_sb = const.tile([8, 8], f32, tag="eye")
                nc.scalar.dma_start(eye_sb[:], eye_d[:])
                mask_sb = const.tile([64, BPC], bf16, tag="mask")
                nc.scalar.dma_start(mask_sb[:], mask_d[:])
                psum_part = psum.tile([BPC, HID], f32, tag="part")
                for w in range(NWIN):
                    g = gat.tile([128, 1, HID], bf16, tag=f"g{w}")
                    nc.gpsimd.dma_gather(
                        g[:],
                        w1_d[w * WIN_ROWS : (w + 1) * WIN_ROWS, :],
                        idx[:, w * IDXC : (w + 1) * IDXC],
                        NVALID,
                        NVALID,
                        HID,
                        queue_num=w % NQ,
                    )
                    nc.tensor.matmul(
                        psum_part[:],
                        mask_sb[:],
                        g[0:64, 0, :],
                        start=(w == 0),
                        stop=(w == NWIN - 1),
                    )
                h_sb = work.tile([BPC, HID], f32, tag="h")
                nc.vector.tensor_copy(h_sb[:], psum_part[:])
                eye_sb_ = eye_sb
                hTr = work.tile([128, 2, BPC], bf16, tag="hTr")
                for c in range(2):
                    p_hT = psum.tile([128, BPC], f32, tag=f"p_hT{c}")
                    nc.tensor.transpose(
                        p_hT[:], h_sb[:, c * 128 : (c + 1) * 128], eye_sb_[:]
                    )
                    nc.scalar.activation(
                        hTr[:, c, :], p_hT[:], Relu, bias=b1_sb[:, c : c + 1]
                    )

            # ---- tail: h2T = relu(W2T-chunks @ hTr + b2T), out = h2 @ W3 + b3
            h2Tr = work.tile([128, 2, BPC], bf16, tag="h2Tr")
            for c in range(2):
                p_h2T = psum.tile([128, BPC], f32, tag=f"p_h2T{c}")
                nc.tensor.matmul(
                    p_h2T[:], w2lh_sb[:, 0, c, :], hTr[:, 0, :], start=True, stop=False
                )
                nc.tensor.matmul(
                    p_h2T[:], w2lh_sb[:, 1, c, :], hTr[:, 1, :], start=False, stop=True
                )
                nc.scalar.activation(
                    h2Tr[:, c, :], p_h2T[:], Relu, bias=b2_sb[:, c : c + 1]
                )
            p_o = psum.tile([BPC, OUT], f32, tag="p_o")
            nc.tensor.matmul(p_o[:], h2Tr[:, 0, :], w3_sb[:, 0, :], start=True, stop=False)
            nc.tensor.matmul(p_o[:], h2Tr[:, 1, :], w3_sb[:, 1, :], start=False, stop=False)
            nc.tensor.matmul(p_o[:], ones8[:], b3_sb[:], start=False, stop=True)
            out_sb = work.tile([BPC, OUT], f32, tag="out_sb")
            nc.vector.tensor_copy(out_sb[:], p_o[:])
            nc.sync.dma_start(out_d[:], out_sb[:])

    nc.compile()
    return nc


def get_nc():
    if "nc" not in _CACHE:
        _CACHE["nc"] = _build_nc()
    return _CACHE["nc"]


def make_in_maps(gate_seq, W1, b1, W2, b2, W3, b3):
    """Shard/marshal the full inputs into per-core input maps (values untouched:
    pure slicing, transposition, retyping and tiling)."""
    gate_seq = np.asarray(gate_seq)
    import ml_dtypes

    W1 = np.ascontiguousarray(np.asarray(W1).astype(ml_dtypes.bfloat16))
    W2 = np.asarray(W2, dtype=np.float32)
    W3 = np.ascontiguousarray(np.asarray(W3).astype(ml_dtypes.bfloat16))
    b1 = np.asarray(b1, dtype=np.float32)
    b2 = np.asarray(b2, dtype=np.float32)
    b3 = np.asarray(b3, dtype=np.float32)

    # W2 chunked for lhsT use: w2lh[p, kc, nc, f] = W2[kc*128 + p, nc*128 + f]
    w2lh = np.ascontiguousarray(
        W2.reshape(2, 128, 2, 128).transpose(1, 0, 2, 3).astype(ml_dtypes.bfloat16)
    )
    b1t = np.ascontiguousarray(b1.reshape(2, 128).T)  # b1t[p, c] = b1[c*128 + p]
    b2t = np.ascontiguousarray(b2.reshape(2, 128).T)
    b3r = np.ascontiguousarray(b3[None, :].astype(ml_dtypes.bfloat16))

    # index-layout permutation (see module docstring)
    p16 = np.arange(16)[:, None]                     # [16, 1]
    col = np.arange(NWIN * IDXC)[None, :]            # [1, NWIN*IDXC]
    i = (col % IDXC) * 16 + p16                      # [16, NWIN*IDXC]
    if TRANSPOSE_GATHER:
        valid = i < NVALID
        b_idx = np.where(valid, i // WIN_POS, 0)
        t_idx = np.where(valid, (col // IDXC) * WIN_POS + i % WIN_POS, 0)
    else:
        valid = np.broadcast_to(True, i.shape)
        b_idx = i % BPC
        t_idx = (col // IDXC) * WIN_POS + i // BPC

    in_maps = []
    for m in range(NCORES):
        gs = gate_seq[m * BPC : (m + 1) * BPC, :]    # [8, 256]
        A = gs[b_idx, t_idx].astype(np.int16)        # [16, NWIN*IDXC]
        A = np.where(valid, A, np.int16(-1))
        gate_prep = np.ascontiguousarray(np.tile(A, (8, 1)))  # [128, NWIN*IDXC]
        in_maps.append(
            {
                "gate_prep": gate_prep,
                "w1": W1,
                "w2lh": w2lh,
                "w3": W3,
                "b1t": b1t,
                "b2t": b2t,
                "b3": b3r,
            }
        )
    return in_maps


def run(inputs, trace=False, **spmd_kwargs):
    from concourse.bass_utils import run_bass_kernel_spmd

    nc = get_nc()
    in_maps = make_in_maps(**inputs)
    res = run_bass_kernel_spmd(
        nc, in_maps, core_ids=list(range(NCORES)), trace=trace, **spmd_kwargs
    )
    out = np.concatenate([r["out"] for r in res.results], axis=0)
    return out, res


def kernel(**inputs) -> np.ndarray:
    out, _ = run(inputs, trace=False)
    return out


# revision 10
# speedup vs baseline: 1.2058x; 1.0070x over previous
"""Trainium2 Bass kernel for nn_Encoder_MLP (embedding gather + sum + 2-layer MLP tail).

Reference computation:
    x = where(gate_seq < 0, A, gate_seq)            # [B, T]   (inputs never negative)
    Wr = W1.reshape(T, V, HID)
    h  = Wr[arange(T)[None,:], x].sum(axis=1) + b1  # [B, HID]  gather B*T rows, sum over T
    h  = relu(h); h = relu(h @ W2 + b2); out = h @ W3 + b3

Sharding (8 cores): data-parallel over the batch axis, W1 fully replicated
(bf16, 512 MB/core in HBM). Core m owns batches [8m, 8m+8) and gathers all
T=256 positions for them. NO collective: the T-sharded variant's [64,256]
ReduceScatter cost 95+us of barrier/ncfw/RS on a 115-137us exec (the
collectives-init barrier absorbs cross-core NEFF launch skew), while the
whole batch-parallel kernel runs in ~41us without it.

Pair-gather: dma_gather idx are int16 (<=32768 addressable elements), so at
row granularity one call's window is 8 positions x 4096 vocab and a core
needs 32 calls x 64 idx; the measured per-call cost (~1.4us fixed + ~8ns/idx,
4 SWDGE queues) makes that a ~15us desc-gen phase. Instead gather at 2-ROW
block granularity: in_ap views the window as [32768, 512] (pairs of rows),
idx = (j*4096 + x) >> 1 = j*2048 + (x>>1) addresses 16-position windows
(65536 rows), so 16 calls x 128 idx halve the fixed-cost total (~10us phase,
2 MB/core gathered instead of 1 MB). Each gathered 512-elem block holds
[row r&~1 | row r|1]; the wanted half is picked by parity = x & 1 via
  h = bmask^T . SUM_w even_w  +  SUM_w om_w^T . (odd_w - even_w)
where bmask[p,b] = (p%8 == b) and om_w[p,:] = bmask[p,:] * parity[p,w]:
16 DVE subs (d_w) + 15 DVE fold adds (even sum) + 16+1 accumulating PE
matmuls, all hidden under the gather phase (DVE ~9us busy, PE ~7us busy).
parity comes from gate_T, a second value-independent host permutation of
gate_seq ([128,16], partition-major), via bitwise_and + bf16 copy on device.

Index layout: call w (queue w%4, slot w//4) gathers 128 blocks to partitions
p = i = j*8 + b (j = position-in-window in [0,16), b = batch-in-core), so
partition p holds batch p%8. idx list position i lives at
idx_tile[i%16, w*8 + i//16] (16-partition wrap, replicated x8 for the 8 Q7
cores). The j*2048 rebase is an inline const added on device; the x>>1 is a
device tensor_scalar shift; the host only permutes/retypes gate_seq.

Tail MLP (per-core [8,256], no second transpose): PE-transpose h ->
relu(+b1T) -> 4 matmuls with W2 128x128 chunks as lhsT (output already
transposed) -> relu(+b2T) -> 2 matmuls + rank-1 bias matmul -> [8,256].
Host concatenates the per-core outputs.
"""

import sys

import numpy as np

if "/opt/trn_rl_repo" not in sys.path:
    sys.path.insert(0, "/opt/trn_rl_repo")

B = 64
T = 256
V = 4096
HID = 256
OUT = 256
NCORES = 8
BPC = B // NCORES          # batches per core = 8
WIN_POS = 16               # positions per gather window (2-row blocks: 32768 int16-addressable)
NWIN = T // WIN_POS        # 16 windows per core
WIN_ROWS = WIN_POS * V     # 65536 rows = 32768 blocks
NIDX = BPC * WIN_POS       # 128 indices per window
IDXC = NIDX // 16          # 8 idx columns per window
NQ = 4                     # SWDGE queues

_CACHE = {}


def _host_consts():
    import ml_dtypes

    # idx list position i = (col % IDXC)*16 + p%16; j = i//8 -> rebase j*2048
    p = np.arange(128)[:, None]
    col = np.arange(NWIN * IDXC)[None, :]
    i = (col % IDXC) * 16 + (p % 16)
    ubias = np.ascontiguousarray(
        np.broadcast_to((i // BPC) * (V // 2), (128, NWIN * IDXC)).astype(np.int16)
    )
    # bmask[p, b] = 1 if p % 8 == b
    bmask = (np.arange(128)[:, None] % BPC == np.arange(BPC)[None, :]).astype(
        ml_dtypes.bfloat16
    )
    eye8 = np.eye(8, dtype=np.float32)
    return ubias, np.ascontiguousarray(bmask), eye8


def _build_nc():
    import concourse.bacc as bacc
    import concourse.mybir as mybir
    import concourse.tile as tile

    f32 = mybir.dt.float32
    bf16 = mybir.dt.bfloat16
    i16 = mybir.dt.int16
    Relu = mybir.ActivationFunctionType.Relu
    add = mybir.AluOpType.add
    sub = mybir.AluOpType.subtract
    mult = mybir.AluOpType.mult
    shr = mybir.AluOpType.logical_shift_right
    band = mybir.AluOpType.bitwise_and

    ubias_np, bmask_np, eye8_np = _host_consts()

    nc = bacc.Bacc(
        "TRN2",
        target_bir_lowering=False,
        debug=False,
        num_devices=NCORES,
        num_swdge_queues=NQ,
    )

    gate_prep_d = nc.dram_tensor(
        "gate_prep", [128, NWIN * IDXC], i16, kind="ExternalInput"
    )
    gate_t_d = nc.dram_tensor("gate_t", [128, NWIN], i16, kind="ExternalInput")
    w1_d = nc.dram_tensor("w1", [T * V, HID], bf16, kind="ExternalInput")
    w2lh_d = nc.dram_tensor("w2lh", [128, 2, 2, 128], bf16, kind="ExternalInput")
    w3_d = nc.dram_tensor("w3", [HID, OUT], bf16, kind="ExternalInput")
    b1_d = nc.dram_tensor("b1t", [128, 2], f32, kind="ExternalInput")
    b2_d = nc.dram_tensor("b2t", [128, 2], f32, kind="ExternalInput")
    b3_d = nc.dram_tensor("b3", [1, OUT], bf16, kind="ExternalInput")
    out_d = nc.dram_tensor("out", [BPC, OUT], f32, kind="ExternalOutput")

    ubias_d = nc.inline_tensor(ubias_np, name="ubias_const")
    bmask_d = nc.inline_tensor(bmask_np, name="bmask_const")
    eye_d = nc.inline_tensor(eye8_np, name="eye_const")

    # Issue the mlp ucode library load before any Tile-scheduled work so the
    # ~10us Q7 library fetch overlaps the NEFF prologue instead of stalling
    # the first dma_gather until ~16us.
    from concourse import library_config

    nc.gpsimd.load_library(library_config.mlp)

    with tile.TileContext(nc) as tc:
        with (
            tc.tile_pool(name="const", bufs=1) as const,
            tc.tile_pool(name="gat", bufs=1) as gat,
            tc.tile_pool(name="work", bufs=2) as work,
            tc.tile_pool(name="psum", bufs=1, space="PSUM") as psum,
        ):
            # ---- critical path: indices (x>>1 then +j*2048) ----
            gp = const.tile([128, NWIN * IDXC], i16, tag="gp")
            nc.sync.dma_start(gp[:], gate_prep_d[:])
            ub = const.tile([128, NWIN * IDXC], i16, tag="ub")
            nc.sync.dma_start(ub[:], ubias_d[:])
            sh = const.tile([128, NWIN * IDXC], i16, tag="sh")
            nc.vector.tensor_scalar(sh[:], gp[:], 1, None, shr)
            idx = const.tile([128, NWIN * IDXC], i16, tag="idx")
            nc.vector.tensor_tensor(idx[:], sh[:], ub[:], add)

            # ---- parity masks (DVE; deps: gate_t DMA only) ----
            gt = const.tile([128, NWIN], i16, tag="gt")
            nc.sync.dma_start(gt[:], gate_t_d[:])
            bmask_sb = const.tile([128, BPC], bf16, tag="bmask")
            nc.scalar.dma_start(bmask_sb[:], bmask_d[:])
            par_i = const.tile([128, NWIN], i16, tag="par_i")
            nc.vector.tensor_scalar(par_i[:], gt[:], 1, None, band)
            parf = const.tile([128, NWIN], f32, tag="parf")
            nc.vector.tensor_copy(parf[:], par_i[:])
            om = const.tile([128, NWIN, BPC], bf16, tag="om")
            for w in range(NWIN):
                nc.vector.tensor_scalar(
                    om[:, w, :], bmask_sb[:], parf[:, w : w + 1], None, mult
                )

            # ---- consts / weights preload ----
            eye_sb = const.tile([8, 8], f32, tag="eye")
            nc.scalar.dma_start(eye_sb[:], eye_d[:])
            w2lh_sb = const.tile([128, 2, 2, 128], bf16, tag="w2lh")
            nc.scalar.dma_start(w2lh_sb[:], w2lh_d[:])
            w3_sb = const.tile([128, 2, OUT], bf16, tag="w3")
            nc.scalar.dma_start(w3_sb[:], w3_d[:, :].rearrange("(k p) n -> p k n", p=128))
            b1_sb = const.tile([128, 2], f32, tag="b1")
            nc.scalar.dma_start(b1_sb[:], b1_d[:])
            b2_sb = const.tile([128, 2], f32, tag="b2")
            nc.scalar.dma_start(b2_sb[:], b2_d[:])
            b3_sb = const.tile([1, OUT], bf16, tag="b3")
            nc.scalar.dma_start(b3_sb[:], b3_d[:])
            ones8 = const.tile([1, BPC], bf16, tag="ones8")
            nc.vector.memset(ones8[:], 1.0)

            # ---- pair-gathers + parity-select reduce ----
            g_tiles = []
            for q in range(NQ):
                g = gat.tile([128, NWIN // NQ, 2 * HID], bf16, tag=f"g{q}")
                g_tiles.append(g)

            def ev(w):
                return g_tiles[w % NQ][:, w // NQ, 0:HID]

            def od(w):
                return g_tiles[w % NQ][:, w // NQ, HID : 2 * HID]

            d = work.tile([128, NWIN, HID], bf16, tag="d")
            e1 = work.tile([128, NWIN // 2, HID], bf16, tag="e1")
            psum_part = psum.tile([BPC, HID], f32, tag="part")
            for w in range(NWIN):
                q, k = w % NQ, w // NQ
                win = w1_d[w * WIN_ROWS : (w + 1) * WIN_ROWS, :].rearrange(
                    "(a two) n -> a (two n)", two=2
                )
                nc.gpsimd.dma_gather(
                    g_tiles[q][:, k : k + 1, :],
                    win,
                    idx[:, w * IDXC : (w + 1) * IDXC],
                    NIDX,
                    NIDX,
                    2 * HID,
                    queue_num=q,
                )
                # d_w = odd - even; psum += om_w^T @ d_w   (selection term)
                nc.vector.tensor_tensor(d[:, w, :], od(w), ev(w), sub)
                nc.tensor.matmul(
                    psum_part[:], om[:, w, :], d[:, w, :], start=(w == 0), stop=False
                )
                if w % 2 == 1:
                    nc.vector.tensor_tensor(e1[:, w // 2, :], ev(w - 1), ev(w), add)
            # fold the 8 even-pair sums -> even_sum, add via bmask matmul
            e2 = work.tile([128, NWIN // 4, HID], bf16, tag="e2")
            nc.vector.tensor_add(e2[:], e1[:, 0 : NWIN // 4, :], e1[:, NWIN // 4 :, :])
            e3 = work.tile([128, NWIN // 8, HID], bf16, tag="e3")
            nc.vector.tensor_add(e3[:], e2[:, 0 : NWIN // 8, :], e2[:, NWIN // 8 :, :])
            e4 = work.tile([128, HID], bf16, tag="e4")
            nc.vector.tensor_add(e4[:], e3[:, 0, :], e3[:, 1, :])
            nc.tensor.matmul(psum_part[:], bmask_sb[:], e4[:], start=False, stop=True)

            h_sb = work.tile([BPC, HID], f32, tag="h")
            nc.vector.tensor_copy(h_sb[:], psum_part[:])

            # ---- tail MLP on [8, 256] shard ----
            hTr = work.tile([128, 2, BPC], bf16, tag="hTr")
            for c in range(2):
                p_hT = psum.tile([128, BPC], f32, tag=f"p_hT{c}")
                nc.tensor.transpose(
                    p_hT[:], h_sb[:, c * 128 : (c + 1) * 128], eye_sb[:]
                )
                nc.scalar.activation(
                    hTr[:, c, :], p_hT[:], Relu, bias=b1_sb[:, c : c + 1]
                )
            h2Tr = work.tile([128, 2, BPC], bf16, tag="h2Tr")
            for c in range(2):
                p_h2T = psum.tile([128, BPC], f32, tag=f"p_h2T{c}")
                nc.tensor.matmul(
                    p_h2T[:], w2lh_sb[:, 0, c, :], hTr[:, 0, :], start=True, stop=False
                )
                nc.tensor.matmul(
                    p_h2T[:], w2lh_sb[:, 1, c, :], hTr[:, 1, :], start=False, stop=True
                )
                nc.scalar.activation(
                    h2Tr[:, c, :], p_h2T[:], Relu, bias=b2_sb[:, c : c + 1]
                )
            p_o = psum.tile([BPC, OUT], f32, tag="p_o")
            nc.tensor.matmul(p_o[:], h2Tr[:, 0, :], w3_sb[:, 0, :], start=True, stop=False)
            nc.tensor.matmul(p_o[:], h2Tr[:, 1, :], w3_sb[:, 1, :], start=False, stop=False)
            nc.tensor.matmul(p_o[:], ones8[:], b3_sb[:], start=False, stop=True)
            out_sb = work.tile([BPC, OUT], f32, tag="out_sb")
            nc.vector.tensor_copy(out_sb[:], p_o[:])
            nc.sync.dma_start(out_d[:], out_sb[:])

    nc.compile()
    return nc


def get_nc():
    if "nc" not in _CACHE:
        _CACHE["nc"] = _build_nc()
    return _CACHE["nc"]


def make_in_maps(gate_seq, W1, b1, W2, b2, W3, b3):
    """Shard/marshal the full inputs into per-core input maps (values untouched:
    pure slicing, transposition, retyping and tiling)."""
    gate_seq = np.asarray(gate_seq)
    import ml_dtypes

    W1 = np.ascontiguousarray(np.asarray(W1).astype(ml_dtypes.bfloat16))
    W2 = np.asarray(W2, dtype=np.float32)
    W3 = np.ascontiguousarray(np.asarray(W3).astype(ml_dtypes.bfloat16))
    b1 = np.asarray(b1, dtype=np.float32)
    b2 = np.asarray(b2, dtype=np.float32)
    b3 = np.asarray(b3, dtype=np.float32)

    # W2 chunked for lhsT use: w2lh[p, kc, nc, f] = W2[kc*128 + p, nc*128 + f]
    w2lh = np.ascontiguousarray(
        W2.reshape(2, 128, 2, 128).transpose(1, 0, 2, 3).astype(ml_dtypes.bfloat16)
    )
    b1t = np.ascontiguousarray(b1.reshape(2, 128).T)  # b1t[p, c] = b1[c*128 + p]
    b2t = np.ascontiguousarray(b2.reshape(2, 128).T)
    b3r = np.ascontiguousarray(b3[None, :].astype(ml_dtypes.bfloat16))

    # index-layout permutation (see module docstring)
    p16 = np.arange(16)[:, None]                     # [16, 1]
    col = np.arange(NWIN * IDXC)[None, :]            # [1, NWIN*IDXC]
    i = (col % IDXC) * 16 + p16                      # [16, NWIN*IDXC]
    b_idx = i % BPC
    t_idx = (col // IDXC) * WIN_POS + i // BPC
    # gate_T[p, w] = gate_seq[8m + p%8, w*16 + p//8]
    pp = np.arange(128)[:, None]
    ww = np.arange(NWIN)[None, :]
    bt_idx = np.broadcast_to(pp % BPC, (128, NWIN))
    tt_idx = ww * WIN_POS + pp // BPC

    in_maps = []
    for m in range(NCORES):
        gs = gate_seq[m * BPC : (m + 1) * BPC, :]    # [8, 256]
        A = gs[b_idx, t_idx].astype(np.int16)        # [16, NWIN*IDXC]
        gate_prep = np.ascontiguousarray(np.tile(A, (8, 1)))  # [128, NWIN*IDXC]
        gate_t = np.ascontiguousarray(gs[bt_idx, tt_idx].astype(np.int16))  # [128, NWIN]
        in_maps.append(
            {
                "gate_prep": gate_prep,
                "gate_t": gate_t,
                "w1": W1,
                "w2lh": w2lh,
                "w3": W3,
                "b1t": b1t,
                "b2t": b2t,
                "b3": b3r,
            }
        )
    return in_maps


def run(inputs, trace=False, **spmd_kwargs):
    from concourse.bass_utils import run_bass_kernel_spmd

    nc = get_nc()
    in_maps = make_in_maps(**inputs)
    res = run_bass_kernel_spmd(
        nc, in_maps, core_ids=list(range(NCORES)), trace=trace, **spmd_kwargs
    )
    out = np.concatenate([r["out"] for r in res.results], axis=0)
    return out, res


def kernel(**inputs) -> np.ndarray:
    out, _ = run(inputs, trace=False)
    return out


# revision 11
# speedup vs baseline: 1.2792x; 1.0609x over previous
"""Trainium2 Bass kernel for nn_Encoder_MLP (embedding gather + sum + 2-layer MLP tail).

Reference computation:
    x = where(gate_seq < 0, A, gate_seq)            # [B, T]   (inputs never negative)
    Wr = W1.reshape(T, V, HID)
    h  = Wr[arange(T)[None,:], x].sum(axis=1) + b1  # [B, HID]  gather B*T rows, sum over T
    h  = relu(h); h = relu(h @ W2 + b2); out = h @ W3 + b3

Sharding (8 cores): data-parallel over the batch axis, W1 fully replicated
(bf16, 512 MB/core in HBM). Core m owns batches [8m, 8m+8) and gathers all
T=256 positions for them. NO collective: the T-sharded variant's [64,256]
ReduceScatter cost 95+us of barrier/ncfw/RS on a 115-137us exec (the
collectives-init barrier absorbs cross-core NEFF launch skew), while the
whole batch-parallel kernel runs in ~41us without it.

Pair-gather: dma_gather idx are int16 (<=32768 addressable elements), so at
row granularity one call's window is 8 positions x 4096 vocab and a core
needs 32 calls x 64 idx; the measured per-call cost (~1.4us fixed + ~8ns/idx,
4 SWDGE queues) makes that a ~15us desc-gen phase. Instead gather at 2-ROW
block granularity: in_ap views the window as [32768, 512] (pairs of rows),
idx = (j*4096 + x) >> 1 = j*2048 + (x>>1) addresses 16-position windows
(65536 rows), so 16 calls x 128 idx halve the per-call fixed-cost total.
Measured: the phase is then DMA-bound instead (2 MB/core over 4 SWDGE
queues at ~31-40 GB/s/queue ~= 14us, vs the 32-call variant's 15.2us
desc-gen bound with 1 MB hidden under it) - 41.1us vs 41.4us total, a wash;
both are at this gather engine's floor. Each gathered 512-elem block holds
[row r&~1 | row r|1]; the wanted half is picked by parity = x & 1 via
  h = bmask^T . SUM_w even_w  +  SUM_w om_w^T . (odd_w - even_w)
where bmask[p,b] = (p%8 == b) and om_w[p,:] = bmask[p,:] * parity[p,w]:
16 DVE subs (d_w) + 15 DVE fold adds (even sum) + 16+1 accumulating PE
matmuls, all hidden under the gather phase (DVE ~9us busy, PE ~7us busy).
parity comes from gate_T, a second value-independent host permutation of
gate_seq ([128,16], partition-major), via bitwise_and + bf16 copy on device.

Index layout: call w (queue w%4, slot w//4) gathers 128 blocks to partitions
p = i = j*8 + b (j = position-in-window in [0,16), b = batch-in-core), so
partition p holds batch p%8. idx list position i lives at
idx_tile[i%16, w*8 + i//16] (16-partition wrap, replicated x8 for the 8 Q7
cores). The j*2048 rebase is an inline const added on device; the x>>1 is a
device tensor_scalar shift; the host only permutes/retypes gate_seq.

Tail MLP (per-core [8,256], no second transpose): PE-transpose h ->
relu(+b1T) -> 4 matmuls with W2 128x128 chunks as lhsT (output already
transposed) -> relu(+b2T) -> 2 matmuls + rank-1 bias matmul -> [8,256].
Host concatenates the per-core outputs.
"""

import sys

import numpy as np

if "/opt/trn_rl_repo" not in sys.path:
    sys.path.insert(0, "/opt/trn_rl_repo")

B = 64
T = 256
V = 4096
HID = 256
OUT = 256
NCORES = 8
BPC = B // NCORES          # batches per core = 8
WIN_POS = 16               # positions per gather window (2-row blocks: 32768 int16-addressable)
NWIN = T // WIN_POS        # 16 windows per core
WIN_ROWS = WIN_POS * V     # 65536 rows = 32768 blocks
NIDX = BPC * WIN_POS       # 128 indices per window
IDXC = NIDX // 16          # 8 idx columns per window
NQ = 4                     # SWDGE queues

_CACHE = {}


def _host_consts():
    import ml_dtypes

    # idx list position i = (col % IDXC)*16 + p%16; j = i//8 -> rebase j*2048
    p = np.arange(128)[:, None]
    col = np.arange(NWIN * IDXC)[None, :]
    i = (col % IDXC) * 16 + (p % 16)
    ubias = np.ascontiguousarray(
        np.broadcast_to((i // BPC) * (V // 2), (128, NWIN * IDXC)).astype(np.int16)
    )
    # bmask[p, b] = 1 if p % 8 == b
    bmask = (np.arange(128)[:, None] % BPC == np.arange(BPC)[None, :]).astype(
        ml_dtypes.bfloat16
    )
    eye8 = np.eye(8, dtype=np.float32)
    return ubias, np.ascontiguousarray(bmask), eye8


def _build_nc():
    import concourse.bacc as bacc
    import concourse.mybir as mybir
    import concourse.tile as tile

    f32 = mybir.dt.float32
    bf16 = mybir.dt.bfloat16
    i16 = mybir.dt.int16
    Relu = mybir.ActivationFunctionType.Relu
    add = mybir.AluOpType.add
    sub = mybir.AluOpType.subtract
    mult = mybir.AluOpType.mult
    shr = mybir.AluOpType.logical_shift_right
    band = mybir.AluOpType.bitwise_and

    ubias_np, bmask_np, eye8_np = _host_consts()

    nc = bacc.Bacc(
        "TRN2",
        target_bir_lowering=False,
        debug=False,
        num_devices=NCORES,
        num_swdge_queues=NQ,
    )

    gate_prep_d = nc.dram_tensor(
        "gate_prep", [128, NWIN * IDXC], i16, kind="ExternalInput"
    )
    gate_t_d = nc.dram_tensor("gate_t", [128, NWIN], i16, kind="ExternalInput")
    w1_d = nc.dram_tensor("w1", [T * V, HID], bf16, kind="ExternalInput")
    w2lh_d = nc.dram_tensor("w2lh", [128, 2, 2, 128], bf16, kind="ExternalInput")
    w3_d = nc.dram_tensor("w3", [HID, OUT], bf16, kind="ExternalInput")
    b1_d = nc.dram_tensor("b1t", [128, 2], f32, kind="ExternalInput")
    b2_d = nc.dram_tensor("b2t", [128, 2], f32, kind="ExternalInput")
    b3_d = nc.dram_tensor("b3", [1, OUT], bf16, kind="ExternalInput")
    out_d = nc.dram_tensor("out", [BPC, OUT], f32, kind="ExternalOutput")

    ubias_d = nc.inline_tensor(ubias_np, name="ubias_const")
    bmask_d = nc.inline_tensor(bmask_np, name="bmask_const")
    eye_d = nc.inline_tensor(eye8_np, name="eye_const")

    # Issue the mlp ucode library load before any Tile-scheduled work so the
    # ~10us Q7 library fetch overlaps the NEFF prologue instead of stalling
    # the first dma_gather until ~16us.
    from concourse import library_config

    nc.gpsimd.load_library(library_config.mlp)

    with tile.TileContext(nc) as tc:
        with (
            tc.tile_pool(name="const", bufs=1) as const,
            tc.tile_pool(name="gat", bufs=1) as gat,
            tc.tile_pool(name="work", bufs=2) as work,
            tc.tile_pool(name="psum", bufs=1, space="PSUM") as psum,
        ):
            # ---- critical path: indices (x>>1 then +j*2048) ----
            gp = const.tile([128, NWIN * IDXC], i16, tag="gp")
            nc.sync.dma_start(gp[:], gate_prep_d[:])
            ub = const.tile([128, NWIN * IDXC], i16, tag="ub")
            nc.sync.dma_start(ub[:], ubias_d[:])
            sh = const.tile([128, NWIN * IDXC], i16, tag="sh")
            nc.vector.tensor_scalar(sh[:], gp[:], 1, None, shr)
            idx = const.tile([128, NWIN * IDXC], i16, tag="idx")
            nc.vector.tensor_tensor(idx[:], sh[:], ub[:], add)

            # ---- parity masks (DVE; deps: gate_t DMA only) ----
            gt = const.tile([128, NWIN], i16, tag="gt")
            nc.sync.dma_start(gt[:], gate_t_d[:])
            bmask_sb = const.tile([128, BPC], bf16, tag="bmask")
            nc.scalar.dma_start(bmask_sb[:], bmask_d[:])
            par_i = const.tile([128, NWIN], i16, tag="par_i")
            nc.vector.tensor_scalar(par_i[:], gt[:], 1, None, band)
            parf = const.tile([128, NWIN], f32, tag="parf")
            nc.vector.tensor_copy(parf[:], par_i[:])
            om = const.tile([128, NWIN, BPC], bf16, tag="om")
            for w in range(NWIN):
                nc.vector.tensor_scalar(
                    om[:, w, :], bmask_sb[:], parf[:, w : w + 1], None, mult
                )

            # ---- consts / weights preload ----
            eye_sb = const.tile([8, 8], f32, tag="eye")
            nc.scalar.dma_start(eye_sb[:], eye_d[:])
            w2lh_sb = const.tile([128, 2, 2, 128], bf16, tag="w2lh")
            nc.scalar.dma_start(w2lh_sb[:], w2lh_d[:])
            w3_sb = const.tile([128, 2, OUT], bf16, tag="w3")
            nc.scalar.dma_start(w3_sb[:], w3_d[:, :].rearrange("(k p) n -> p k n", p=128))
            b1_sb = const.tile([128, 2], f32, tag="b1")
            nc.scalar.dma_start(b1_sb[:], b1_d[:])
            b2_sb = const.tile([128, 2], f32, tag="b2")
            nc.scalar.dma_start(b2_sb[:], b2_d[:])
            b3_sb = const.tile([1, OUT], bf16, tag="b3")
            nc.scalar.dma_start(b3_sb[:], b3_d[:])
            ones8 = const.tile([1, BPC], bf16, tag="ones8")
            nc.vector.memset(ones8[:], 1.0)

            # ---- pair-gathers + parity-select reduce ----
            g_tiles = []
            for q in range(NQ):
                g = gat.tile([128, NWIN // NQ, 2 * HID], bf16, tag=f"g{q}")
                g_tiles.append(g)

            def ev(w):
                return g_tiles[w % NQ][:, w // NQ, 0:HID]

            def od(w):
                return g_tiles[w % NQ][:, w // NQ, HID : 2 * HID]

            d = work.tile([128, NWIN, HID], bf16, tag="d")
            e1 = work.tile([128, NWIN // 2, HID], bf16, tag="e1")
            psum_part = psum.tile([BPC, HID], f32, tag="part")
            for w in range(NWIN):
                q, k = w % NQ, w // NQ
                win = w1_d[w * WIN_ROWS : (w + 1) * WIN_ROWS, :].rearrange(
                    "(a two) n -> a (two n)", two=2
                )
                nc.gpsimd.dma_gather(
                    g_tiles[q][:, k : k + 1, :],
                    win,
                    idx[:, w * IDXC : (w + 1) * IDXC],
                    NIDX,
                    NIDX,
                    2 * HID,
                    queue_num=q,
                )
                # d_w = odd - even; psum += om_w^T @ d_w   (selection term)
                nc.vector.tensor_tensor(d[:, w, :], od(w), ev(w), sub)
                nc.tensor.matmul(
                    psum_part[:], om[:, w, :], d[:, w, :], start=(w == 0), stop=False
                )
                if w % 2 == 1:
                    nc.vector.tensor_tensor(e1[:, w // 2, :], ev(w - 1), ev(w), add)
            # fold the 8 even-pair sums -> even_sum, add via bmask matmul
            e2 = work.tile([128, NWIN // 4, HID], bf16, tag="e2")
            nc.vector.tensor_add(e2[:], e1[:, 0 : NWIN // 4, :], e1[:, NWIN // 4 :, :])
            e3 = work.tile([128, NWIN // 8, HID], bf16, tag="e3")
            nc.vector.tensor_add(e3[:], e2[:, 0 : NWIN // 8, :], e2[:, NWIN // 8 :, :])
            e4 = work.tile([128, HID], bf16, tag="e4")
            nc.vector.tensor_add(e4[:], e3[:, 0, :], e3[:, 1, :])
            nc.tensor.matmul(psum_part[:], bmask_sb[:], e4[:], start=False, stop=True)

            h_sb = work.tile([BPC, HID], f32, tag="h")
            nc.vector.tensor_copy(h_sb[:], psum_part[:])

            # ---- tail MLP on [8, 256] shard ----
            hTr = work.tile([128, 2, BPC], bf16, tag="hTr")
            for c in range(2):
                p_hT = psum.tile([128, BPC], f32, tag=f"p_hT{c}")
                nc.tensor.transpose(
                    p_hT[:], h_sb[:, c * 128 : (c + 1) * 128], eye_sb[:]
                )
                nc.scalar.activation(
                    hTr[:, c, :], p_hT[:], Relu, bias=b1_sb[:, c : c + 1]
                )
            h2Tr = work.tile([128, 2, BPC], bf16, tag="h2Tr")
            for c in range(2):
                p_h2T = psum.tile([128, BPC], f32, tag=f"p_h2T{c}")
                nc.tensor.matmul(
                    p_h2T[:], w2lh_sb[:, 0, c, :], hTr[:, 0, :], start=True, stop=False
                )
                nc.tensor.matmul(
                    p_h2T[:], w2lh_sb[:, 1, c, :], hTr[:, 1, :], start=False, stop=True
                )
                nc.scalar.activation(
                    h2Tr[:, c, :], p_h2T[:], Relu, bias=b2_sb[:, c : c + 1]
                )
            p_o = psum.tile([BPC, OUT], f32, tag="p_o")
            nc.tensor.matmul(p_o[:], h2Tr[:, 0, :], w3_sb[:, 0, :], start=True, stop=False)
            nc.tensor.matmul(p_o[:], h2Tr[:, 1, :], w3_sb[:, 1, :], start=False, stop=False)
            nc.tensor.matmul(p_o[:], ones8[:], b3_sb[:], start=False, stop=True)
            out_sb = work.tile([BPC, OUT], f32, tag="out_sb")
            nc.vector.tensor_copy(out_sb[:], p_o[:])
            nc.sync.dma_start(out_d[:], out_sb[:])

    nc.compile()
    return nc


def get_nc():
    if "nc" not in _CACHE:
        _CACHE["nc"] = _build_nc()
    return _CACHE["nc"]


def make_in_maps(gate_seq, W1, b1, W2, b2, W3, b3):
    """Shard/marshal the full inputs into per-core input maps (values untouched:
    pure slicing, transposition, retyping and tiling)."""
    gate_seq = np.asarray(gate_seq)
    import ml_dtypes

    W1 = np.ascontiguousarray(np.asarray(W1).astype(ml_dtypes.bfloat16))
    W2 = np.asarray(W2, dtype=np.float32)
    W3 = np.ascontiguousarray(np.asarray(W3).astype(ml_dtypes.bfloat16))
    b1 = np.asarray(b1, dtype=np.float32)
    b2 = np.asarray(b2, dtype=np.float32)
    b3 = np.asarray(b3, dtype=np.float32)

    # W2 chunked for lhsT use: w2lh[p, kc, nc, f] = W2[kc*128 + p, nc*128 + f]
    w2lh = np.ascontiguousarray(
        W2.reshape(2, 128, 2, 128).transpose(1, 0, 2, 3).astype(ml_dtypes.bfloat16)
    )
    b1t = np.ascontiguousarray(b1.reshape(2, 128).T)  # b1t[p, c] = b1[c*128 + p]
    b2t = np.ascontiguousarray(b2.reshape(2, 128).T)
    b3r = np.ascontiguousarray(b3[None, :].astype(ml_dtypes.bfloat16))

    # index-layout permutation (see module docstring)
    p16 = np.arange(16)[:, None]                     # [16, 1]
    col = np.arange(NWIN * IDXC)[None, :]            # [1, NWIN*IDXC]
    i = (col % IDXC) * 16 + p16                      # [16, NWIN*IDXC]
    b_idx = i % BPC
    t_idx = (col // IDXC) * WIN_POS + i // BPC
    # gate_T[p, w] = gate_seq[8m + p%8, w*16 + p//8]
    pp = np.arange(128)[:, None]
    ww = np.arange(NWIN)[None, :]
    bt_idx = np.broadcast_to(pp % BPC, (128, NWIN))
    tt_idx = ww * WIN_POS + pp // BPC

    in_maps = []
    for m in range(NCORES):
        gs = gate_seq[m * BPC : (m + 1) * BPC, :]    # [8, 256]
        A = gs[b_idx, t_idx].astype(np.int16)        # [16, NWIN*IDXC]
        gate_prep = np.ascontiguousarray(np.tile(A, (8, 1)))  # [128, NWIN*IDXC]
        gate_t = np.ascontiguousarray(gs[bt_idx, tt_idx].astype(np.int16))  # [128, NWIN]
        in_maps.append(
            {
                "gate_prep": gate_prep,
                "gate_t": gate_t,
                "w1": W1,
                "w2lh": w2lh,
                "w3": W3,
                "b1t": b1t,
                "b2t": b2t,
                "b3": b3r,
            }
        )
    return in_maps


def run(inputs, trace=False, **spmd_kwargs):
    from concourse.bass_utils import run_bass_kernel_spmd

    nc = get_nc()
    in_maps = make_in_maps(**inputs)
    res = run_bass_kernel_spmd(
        nc, in_maps, core_ids=list(range(NCORES)), trace=trace, **spmd_kwargs
    )
    out = np.concatenate([r["out"] for r in res.results], axis=0)
    return out, res


def kernel(**inputs) -> np.ndarray:
    out, _ = run(inputs, trace=False)
    return out
